# revision 20
# baseline (speedup 1.0000x reference)
"""Trainium2 Bass kernel for nn_Bottleneck (QAT bottleneck block), 8-core data parallel.

Strategy
--------
Data-parallel over batch: core c processes images [2c, 2c+1]. The per-channel
activation-quantization scales (delta_k) are global maxima over the WHOLE batch,
so after each stage every core AllGathers its local per-channel absmax ([64] or
[256] floats) and reduces locally; everything else is core-local.

Math (device):
  stage k: PSUM M = conv(wq_k_folded, a_prev)  (PE)
           t = M + beta_k                       (DVE ttr, fused per-channel absmax)
           [AllGather absmax -> delta_k, s_k=1/delta_k]
           v = t*s_k + MAGIC                    (ACT, in-place; MAGIC = 1.5*2^23 forces RNE)
           a = max(v-MAGIC, 0) -> bf16          (GPSIMD, = relu(round(t*s_k)); integer-valued)
  residual: z = q3*d3 + x (fused absmax), u4=relu(z*s4), r4=rne(u4), out=r4*d4.

PE precision: stage1 fp32 (x is raw fp32); stages 2/3 run on integer-valued bf16
activations with the folded fp32 weights split into two bf16 parts (hi/lo).
hi@a and lo@a are packed into ONE K=128 matmul: the weight lhsT holds hi in
partitions 0-63 and lo in 64-127, while the rhs activations are duplicated
across both partition halves. The duplication is free: every conv's lhsT
duplicates its output channels (M=64 -> 128), so PSUM (and everything derived
from it) is born duplicated. Host-side validation: this scheme matches the
jax fp32 reference to relL2 ~6e-4 (the pure-fp32 reordering noise floor is ~5e-4).

Weight folding W' = wq * d_prev[cin] depends on runtime scales, so the fold and
the bf16 hi/lo split run on-device (tiny [64-128 x <=1152] DVE ops).
"""
import sys
import os

sys.path.insert(0, "/opt/trn_rl_repo")

import numpy as np

import concourse.bacc as bacc
import concourse.bass as bass
import concourse.tile as tile
from concourse import mybir
from concourse.bass_utils import run_bass_kernel_spmd

F32 = np.float32
DT = mybir.dt
NCORES = 8
N, CIN, H, W = 16, 256, 56, 56
PX = H * W            # 3136
HP, WP = H + 2, W + 2  # 58, 58 padded
PXP = HP * WP          # 3364
NB = 7                 # bands of 8 rows
BAND = 8 * W           # 448
MAGIC = float(1.5 * 2 ** 23)
QMAX = F32(127.0)
EPS = F32(1e-5)

AOP = mybir.AluOpType
AF = mybir.ActivationFunctionType


# ----------------------------------------------------------------------------- host prep
def _host_fold(w, g, b, m, v):
    """Replicate reference's quant_w(w*fact) in exact fp32; return wq, beta."""
    fact = (g.astype(F32) / np.sqrt(v.astype(F32) + EPS).astype(F32)).astype(F32)
    ws = (w.astype(F32) * fact[:, None, None, None]).astype(F32)
    delta = np.maximum((np.abs(ws).max(axis=(1, 2, 3), keepdims=True) / QMAX).astype(F32), F32(1e-8))
    wq = (np.clip(np.round((ws / delta).astype(F32)), -127, 127) * delta).astype(F32)
    beta = (b.astype(F32) - m.astype(F32) * fact).astype(F32)
    return wq, beta


def _dup2(a):
    """[64,...] -> [128,...] partition-dup along axis 0."""
    return np.concatenate([a, a], axis=0)


def _build_nc():
    nc = bacc.Bacc("TRN2", target_bir_lowering=False, debug=False, num_devices=NCORES)

    xin = nc.dram_tensor("xin", [2, CIN, PX], DT.float32, kind="ExternalInput")
    w1t = nc.dram_tensor("w1t", [2, 128, 128], DT.float32, kind="ExternalInput")   # [kchunk, cin, cout-dup]
    w2d = nc.dram_tensor("w2d", [128, 9, 128], DT.float32, kind="ExternalInput")   # [cin-dup, tap, cout-dup]
    w3d = nc.dram_tensor("w3d", [128, 2, 128], DT.float32, kind="ExternalInput")   # [cin-dup, couthalf, cout]
    b1d = nc.dram_tensor("b1d", [128], DT.float32, kind="ExternalInput")           # beta1 dup
    b2d = nc.dram_tensor("b2d", [128], DT.float32, kind="ExternalInput")
    b3d = nc.dram_tensor("b3d", [256], DT.float32, kind="ExternalInput")           # beta3 (2 chunks of 128)
    outd = nc.dram_tensor("outp", [2, CIN, PX], DT.float32, kind="ExternalOutput")

    with tile.TileContext(nc) as tc:
        _emit(tc, xin, w1t, w2d, w3d, b1d, b2d, b3d, outd)

    nc.compile()
    return nc


def _emit(tc, xin, w1t, w2d, w3d, b1d, b2d, b3d, outd):
    nc = tc.nc
    rg = [list(range(NCORES))]

    sb = tc.alloc_tile_pool(name="sb", bufs=1)
    big = tc.alloc_tile_pool(name="big", bufs=7)       # 13.5KB f32 staging slots
    bfp = tc.alloc_tile_pool(name="bfp", bufs=7)       # bf16 activation slots
    vec = tc.alloc_tile_pool(name="vec", bufs=1)       # small per-channel vectors
    ps1 = tc.alloc_tile_pool(name="ps1", bufs=2, space="PSUM")
    ps2 = tc.alloc_tile_pool(name="ps2", bufs=2, space="PSUM")
    ps3 = tc.alloc_tile_pool(name="ps3", bufs=2, space="PSUM")
    dram = tc.alloc_tile_pool(name="dram", bufs=1, space="DRAM")

    # ---------------- persistent SBUF loads
    # weights + per-channel consts FIRST (small; the first matmul gates on w1sb)
    w1sb = sb.tile([128, 2, 128], DT.float32, name="w1sb", tag="w1sb")
    nc.sync.dma_start(out=w1sb, in_=w1t.rearrange("k c j -> c k j"))
    b1s = vec.tile([128, 1], DT.float32, name="b1s", tag="b1s")
    nc.sync.dma_start(out=b1s, in_=b1d.rearrange("(c o) -> c o", o=1))
    b2s = vec.tile([128, 1], DT.float32, name="b2s", tag="b2s")
    nc.sync.dma_start(out=b2s, in_=b2d.rearrange("(c o) -> c o", o=1))
    b3s = vec.tile([128, 2], DT.float32, name="b3s", tag="b3s")
    nc.sync.dma_start(out=b3s, in_=b3d.rearrange("(h c) -> c h", c=128))
    w2f = sb.tile([128, 9, 128], DT.float32, name="w2f", tag="w2f")
    nc.sync.dma_start(out=w2f, in_=w2d[:, :, :])
    w3f = sb.tile([128, 2, 128], DT.float32, name="w3f", tag="w3f")
    nc.sync.dma_start(out=w3f, in_=w3d[:, :, :])
    # x in half-image chunks: efficient descriptors, early first-band availability
    xsb = [sb.tile([128, 2, PX], DT.float32, name=f"xsb{k}", tag=f"xsb{k}")
           for k in range(2)]
    HALF = PX // 2
    for i in range(2):
        for h in range(2):
            for k in range(2):
                nc.sync.dma_start(
                    out=xsb[k][:, i, HALF * h:HALF * (h + 1)],
                    in_=xin[i, 128 * k:128 * (k + 1), HALF * h:HALF * (h + 1)])
    magic_t = vec.tile([128, 1], DT.float32, name="magic_t", tag="magic_t")
    nc.vector.memset(magic_t, MAGIC)

    # ---------------- warmup collective: absorbs first-call ncfw cost + core skew
    ccw_i = dram.tile([64], DT.float32, name="ccwi", tag="ccwi")
    ccw_o = dram.tile([64 * NCORES], DT.float32, name="ccwo", tag="ccwo", addr_space="Shared")
    nc.sync.dma_start(out=ccw_i[:], in_=b1d[0:64])
    nc.gpsimd.collective_compute(
        "AllGather", AOP.bypass, replica_groups=rg,
        ins=[ccw_i[:]], outs=[ccw_o[:]],
    )

    # ---------------- collective bounce buffers
    cc_in = [dram.tile([64], DT.float32, name="cc1i", tag="cc1i"),
             dram.tile([64], DT.float32, name="cc2i", tag="cc2i"),
             dram.tile([256], DT.float32, name="cc3i", tag="cc3i"),
             dram.tile([256], DT.float32, name="cc4i", tag="cc4i")]
    cc_out = [dram.tile([64 * NCORES], DT.float32, name="cc1o", tag="cc1o", addr_space="Shared"),
              dram.tile([64 * NCORES], DT.float32, name="cc2o", tag="cc2o", addr_space="Shared"),
              dram.tile([256 * NCORES], DT.float32, name="cc3o", tag="cc3o", addr_space="Shared"),
              dram.tile([256 * NCORES], DT.float32, name="cc4o", tag="cc4o", addr_space="Shared")]

    def allgather_max(idx, mloc, nch, ncol):
        """mloc [128, ncol] local per-channel absmax -> returns (d, s) tiles [128, ncol].

        nch: real channel count (64 -> dup halves, 256 -> 2 chunks)."""
        if nch == 64:
            nc.scalar.dma_start(out=cc_in[idx][:], in_=mloc[0:64, 0:1].rearrange("c o -> (c o)"))
        else:
            nc.scalar.dma_start(out=cc_in[idx].rearrange("(h c) -> c h", c=128), in_=mloc[:, :])
        nc.gpsimd.collective_compute(
            "AllGather", AOP.bypass, replica_groups=rg,
            ins=[cc_in[idx][:]], outs=[cc_out[idx][:]],
        )
        gm = vec.tile([128, ncol, NCORES], DT.float32, name=f"gm{idx}", tag=f"gm{idx}")
        if nch == 64:
            src = cc_out[idx].rearrange("(r o c) -> c o r", c=64, o=1)
            nc.scalar.dma_start(out=gm[0:64], in_=src)
            nc.scalar.dma_start(out=gm[64:128], in_=src)
        else:
            for h in range(2):
                nc.scalar.dma_start(
                    out=gm[:, h, :],
                    in_=cc_out[idx].rearrange("(r h c) -> c h r", c=128, h=2)[:, h, :])
        m = vec.tile([128, ncol], DT.float32, name=f"m{idx}", tag=f"m{idx}")
        nc.vector.reduce_max(out=m, in_=gm, axis=mybir.AxisListType.X)
        d = vec.tile([128, ncol], DT.float32, name=f"d{idx}", tag=f"d{idx}")
        nc.vector.tensor_scalar(out=d, in0=m, scalar1=float(np.float32(1.0) / np.float32(127.0)),
                                scalar2=1e-8, op0=AOP.mult, op1=AOP.max)
        s = vec.tile([128, ncol], DT.float32, name=f"s{idx}", tag=f"s{idx}")
        nc.vector.reciprocal(out=s, in_=d)
        return d, s

    # ================= stage 1: 1x1 conv 256->64(dup), fp32
    t1 = []
    am1 = vec.tile([128, 2], DT.float32, name="am1", tag="am1")
    for i in range(2):
        t1i = big.tile([128, HP, WP], DT.float32, name=f"t1_{i}", tag="bigf32")
        # zero the 1-px border (conv taps read it; ACT/GPSIMD passes keep it 0)
        nc.vector.memset(t1i[:, 0, :], 0.0)
        nc.vector.memset(t1i[:, HP - 1, :], 0.0)
        nc.vector.memset(t1i[:, 1:HP - 1, 0:1], 0.0)
        nc.vector.memset(t1i[:, 1:HP - 1, WP - 1:WP], 0.0)
        for b in range(NB):
            ps = ps1.tile([128, 512], DT.float32, name="ps1t", tag="ps1t")
            for k in range(2):
                nc.tensor.matmul(ps[:, 0:BAND], w1sb[:, k, :],
                                 xsb[k][:, i, BAND * b:BAND * (b + 1)],
                                 start=(k == 0), stop=(k == 1))
            nc.vector.tensor_scalar(
                out=t1i[:, 1 + 8 * b:9 + 8 * b, 1:57],
                in0=ps[:, 0:BAND].rearrange("c (r w) -> c r w", r=8),
                scalar1=b1s, scalar2=None, op0=AOP.add)
        nc.vector.tensor_reduce(out=am1[:, i:i + 1], in_=t1i,
                                axis=mybir.AxisListType.XY, op=AOP.max,
                                apply_absolute_value=True)
        t1.append(t1i)
    m1loc = vec.tile([128, 1], DT.float32, name="m1loc", tag="m1loc")
    nc.vector.reduce_max(out=m1loc, in_=am1, axis=mybir.AxisListType.X)
    d1, s1 = allgather_max(0, m1loc, 64, 1)

    # fold + split stage-2 weights: W2' = w2f * d1[cin]; hi=bf16(W2'), lo=bf16(W2'-hi)
    w2hi = sb.tile([128, 9, 128], DT.bfloat16, name="w2hi", tag="w2hi")
    nc.vector.tensor_scalar(out=w2hi, in0=w2f, scalar1=d1, scalar2=None, op0=AOP.mult)
    p2 = sb.tile([128, 9, 128], DT.bfloat16, name="p2", tag="p2")
    nc.vector.tensor_copy(out=p2[0:64], in_=w2hi[0:64])
    nc.vector.scalar_tensor_tensor(out=p2[64:128], in0=w2f[64:128], scalar=d1[64:128],
                                   in1=w2hi[64:128], op0=AOP.mult, op1=AOP.subtract)

    # a1 = relu(round(t1*s1)) as integer-valued bf16, padded+dup layout
    a1 = []
    for i in range(2):
        nc.scalar.activation(out=t1[i][:], in_=t1[i][:], func=AF.Identity, bias=magic_t, scale=s1)
        a1i = bfp.tile([128, HP, WP], DT.bfloat16, name=f"a1_{i}", tag="bfact")
        nc.vector.tensor_scalar(out=a1i, in0=t1[i][:], scalar1=MAGIC, scalar2=0.0,
                                op0=AOP.subtract, op1=AOP.max)
        a1.append(a1i)

    # ================= stage 2: 3x3 conv 64->64(dup), bf16 hi/lo packed K=128
    t2 = []
    am2 = vec.tile([128, 2], DT.float32, name="am2", tag="am2")
    for i in range(2):
        t2i = big.tile([128, PX], DT.float32, name=f"t2_{i}", tag="bigf32")
        for bt in range(4):  # psum tiles of 2 bands (last has 1)
            bands = [2 * bt, 2 * bt + 1] if bt < 3 else [6]
            ps = ps2.tile([128, 2, 512], DT.float32, name="ps2t", tag="ps2t")
            for tap in range(9):
                dy, dx = tap // 3, tap % 3
                for j, b in enumerate(bands):
                    nc.tensor.matmul(ps[:, j, 0:BAND], p2[:, tap, :],
                                     a1[i][:, 8 * b + dy:8 * b + dy + 8, dx:dx + 56],
                                     start=(tap == 0), stop=(tap == 8))
            for j, b in enumerate(bands):
                nc.vector.tensor_scalar(
                    out=t2i[:, BAND * b:BAND * (b + 1)],
                    in0=ps[:, j, 0:BAND],
                    scalar1=b2s, scalar2=None, op0=AOP.add)
        nc.vector.tensor_reduce(out=am2[:, i:i + 1], in_=t2i,
                                axis=mybir.AxisListType.X, op=AOP.max,
                                apply_absolute_value=True)
        t2.append(t2i)
    m2loc = vec.tile([128, 1], DT.float32, name="m2loc", tag="m2loc")
    nc.vector.reduce_max(out=m2loc, in_=am2, axis=mybir.AxisListType.X)
    d2, s2 = allgather_max(1, m2loc, 64, 1)

    # fold + split stage-3 weights
    w3hi = sb.tile([128, 2, 128], DT.bfloat16, name="w3hi", tag="w3hi")
    nc.vector.tensor_scalar(out=w3hi, in0=w3f, scalar1=d2, scalar2=None, op0=AOP.mult)
    p3 = sb.tile([128, 2, 128], DT.bfloat16, name="p3", tag="p3")
    nc.vector.tensor_copy(out=p3[0:64], in_=w3hi[0:64])
    nc.vector.scalar_tensor_tensor(out=p3[64:128], in0=w3f[64:128], scalar=d2[64:128],
                                   in1=w3hi[64:128], op0=AOP.mult, op1=AOP.subtract)

    a2 = []
    for i in range(2):
        nc.scalar.activation(out=t2[i][:], in_=t2[i][:], func=AF.Identity, bias=magic_t, scale=s2)
        a2i = bfp.tile([128, PX], DT.bfloat16, name=f"a2_{i}", tag="bfact")
        nc.vector.tensor_scalar(out=a2i, in0=t2[i][:], scalar1=MAGIC, scalar2=0.0,
                                op0=AOP.subtract, op1=AOP.max)
        a2.append(a2i)

    # ================= stage 3: 1x1 conv 64->256 (2 chunks of 128), bf16 packed
    t3 = [[None, None], [None, None]]
    am3 = vec.tile([128, 2, 2], DT.float32, name="am3", tag="am3")  # [c, i]
    for i in range(2):
        for c in range(2):
            t3ic = big.tile([128, PX], DT.float32, name=f"t3_{i}{c}", tag="bigf32")
            for b in range(NB):
                ps = ps3.tile([128, 512], DT.float32, name="ps3t", tag="ps3t")
                nc.tensor.matmul(ps[:, 0:BAND], p3[:, c, :],
                                 a2[i][:, BAND * b:BAND * (b + 1)],
                                 start=True, stop=True)
                nc.scalar.activation(out=t3ic[:, BAND * b:BAND * (b + 1)], in_=ps[:, 0:BAND],
                                     func=AF.Identity, bias=b3s[:, c:c + 1], scale=1.0)
            nc.vector.tensor_reduce(out=am3[:, c, i:i + 1], in_=t3ic, axis=mybir.AxisListType.X,
                                    op=AOP.max, apply_absolute_value=True)
            t3[i][c] = t3ic
    m3loc = vec.tile([128, 2], DT.float32, name="m3loc", tag="m3loc")
    nc.vector.reduce_max(out=m3loc, in_=am3, axis=mybir.AxisListType.X)
    d3, s3 = allgather_max(2, m3loc, 256, 2)

    # ================= residual + final quant
    z = [[None, None], [None, None]]
    am4 = vec.tile([128, 2, 2], DT.float32, name="am4", tag="am4")
    for i in range(2):
        for c in range(2):
            t3ic = t3[i][c]
            nc.scalar.activation(out=t3ic[:], in_=t3ic[:], func=AF.Identity,
                                 bias=magic_t, scale=s3[:, c:c + 1])
            q3d = big.tile([128, PX], DT.float32, name=f"q3d_{i}{c}", tag="bigf32")
            nc.vector.tensor_scalar(out=q3d, in0=t3ic, scalar1=MAGIC,
                                    scalar2=d3[:, c:c + 1],
                                    op0=AOP.subtract, op1=AOP.mult)
            zic = q3d
            nc.vector.tensor_add(out=zic[:], in0=q3d[:], in1=xsb[c][:, i, :])
            nc.vector.tensor_reduce(out=am4[:, c, i:i + 1], in_=zic,
                                    axis=mybir.AxisListType.X, op=AOP.max,
                                    apply_absolute_value=True)
            z[i][c] = zic
    m4loc = vec.tile([128, 2], DT.float32, name="m4loc", tag="m4loc")
    nc.vector.reduce_max(out=m4loc, in_=am4, axis=mybir.AxisListType.X)
    d4, s4 = allgather_max(3, m4loc, 256, 2)

    for i in range(2):
        for c in range(2):
            zic = z[i][c]
            nc.scalar.activation(out=zic[:], in_=zic[:], func=AF.Relu,
                                 bias=0.0, scale=s4[:, c:c + 1])
            nc.vector.tensor_scalar(out=zic[:], in0=zic[:], scalar1=MAGIC, scalar2=MAGIC,
                                    op0=AOP.add, op1=AOP.subtract)
            nc.vector.tensor_scalar(out=zic[:], in0=zic[:], scalar1=d4[:, c:c + 1],
                                    scalar2=None, op0=AOP.mult)
            nc.sync.dma_start(out=outd[i, 128 * c:128 * (c + 1), :], in_=zic[:])

    for p in (dram, ps3, ps2, ps1, vec, bfp, big, sb):
        p.release()


_NC_CACHE = {}


def _get_nc():
    if "nc" not in _NC_CACHE:
        _NC_CACHE["nc"] = _build_nc()
    return _NC_CACHE["nc"]


def kernel(x, w1, g1, b1, m1, v1, w2, g2, b2, m2, v2, w3, g3, b3, m3, v3,
           _want_profile=False):
    x = np.ascontiguousarray(x, dtype=F32)

    wq1, beta1 = _host_fold(w1, g1, b1, m1, v1)
    wq2, beta2 = _host_fold(w2, g2, b2, m2, v2)
    wq3, beta3 = _host_fold(w3, g3, b3, m3, v3)

    # stage1 lhsT [kchunk, cin(128), cout-dup(128)]
    w1m = wq1[:, :, 0, 0]                       # [64, 256]
    w1t = np.stack([w1m[:, 0:128].T, w1m[:, 128:256].T], axis=0)       # [2,128,64]
    w1t = np.concatenate([w1t, w1t], axis=2).astype(F32)               # [2,128,128] cout-dup

    # stage2 [cin-dup(128), tap(9), cout-dup(128)]
    w2r = wq2[:, :, :, :].reshape(64, 64, 9).transpose(1, 2, 0)        # [cin, tap, cout]
    w2dn = np.concatenate([w2r, w2r], axis=0)                          # cin-dup
    w2dn = np.concatenate([w2dn, w2dn], axis=2).astype(F32)            # cout-dup

    # stage3 [cin-dup(128), couthalf(2), cout(128)]
    w3r = wq3[:, :, 0, 0].T                                            # [64, 256]
    w3dn = np.stack([w3r[:, 0:128], w3r[:, 128:256]], axis=1)          # [64, 2, 128]
    w3dn = np.concatenate([w3dn, w3dn], axis=0).astype(F32)            # cin-dup

    b1dn = _dup2(beta1).astype(F32)
    b2dn = _dup2(beta2).astype(F32)
    b3dn = beta3.astype(F32)                                           # [256]

    nc = _get_nc()
    in_maps = []
    for c in range(NCORES):
        in_maps.append({
            "xin": np.ascontiguousarray(x[2 * c:2 * c + 2].reshape(2, CIN, PX)),
            "w1t": w1t, "w2d": w2dn, "w3d": w3dn,
            "b1d": b1dn, "b2d": b2dn, "b3d": b3dn,
        })
    res = run_bass_kernel_spmd(nc, in_maps, list(range(NCORES)), trace=_want_profile)
    out = np.empty((N, CIN, PX), dtype=F32)
    for c in range(NCORES):
        out[2 * c:2 * c + 2] = res.results[c]["outp"]
    out = out.reshape(N, CIN, H, W)
    if _want_profile:
        return out, res
    return out


# revision 21
# speedup vs baseline: 1.0117x; 1.0117x over previous
"""Trainium2 Bass kernel for nn_Bottleneck (QAT bottleneck block), 8-core data parallel.

Strategy
--------
Data-parallel over batch: core c processes images [2c, 2c+1]. The per-channel
activation-quantization scales (delta_k) are global maxima over the WHOLE batch,
so after each stage every core AllGathers its local per-channel absmax ([64] or
[256] floats) and reduces locally; everything else is core-local.

Math (device):
  stage k: PSUM M = conv(wq_k_folded, a_prev)  (PE)
           t = M + beta_k                       (DVE ttr, fused per-channel absmax)
           [AllGather absmax -> delta_k, s_k=1/delta_k]
           v = t*s_k + MAGIC                    (ACT, in-place; MAGIC = 1.5*2^23 forces RNE)
           a = max(v-MAGIC, 0) -> bf16          (GPSIMD, = relu(round(t*s_k)); integer-valued)
  residual: z = q3*d3 + x (fused absmax), u4=relu(z*s4), r4=rne(u4), out=r4*d4.

PE precision: stage1 fp32 (x is raw fp32); stages 2/3 run on integer-valued bf16
activations with the folded fp32 weights split into two bf16 parts (hi/lo).
hi@a and lo@a are packed into ONE K=128 matmul: the weight lhsT holds hi in
partitions 0-63 and lo in 64-127, while the rhs activations are duplicated
across both partition halves. The duplication is free: every conv's lhsT
duplicates its output channels (M=64 -> 128), so PSUM (and everything derived
from it) is born duplicated. Host-side validation: this scheme matches the
jax fp32 reference to relL2 ~6e-4 (the pure-fp32 reordering noise floor is ~5e-4).

Weight folding W' = wq * d_prev[cin] depends on runtime scales, so the fold and
the bf16 hi/lo split run on-device (tiny [64-128 x <=1152] DVE ops).
"""
import sys
import os

sys.path.insert(0, "/opt/trn_rl_repo")

import numpy as np

import concourse.bacc as bacc
import concourse.bass as bass
import concourse.tile as tile
from concourse import mybir
from concourse.bass_utils import run_bass_kernel_spmd

F32 = np.float32
DT = mybir.dt
NCORES = 8
N, CIN, H, W = 16, 256, 56, 56
PX = H * W            # 3136
HP, WP = H + 2, W + 2  # 58, 58 padded
PXP = HP * WP          # 3364
NB = 7                 # bands of 8 rows
BAND = 8 * W           # 448
MAGIC = float(1.5 * 2 ** 23)
QMAX = F32(127.0)
EPS = F32(1e-5)

AOP = mybir.AluOpType
AF = mybir.ActivationFunctionType


# ----------------------------------------------------------------------------- host prep
def _host_fold(w, g, b, m, v):
    """Replicate reference's quant_w(w*fact) in exact fp32; return wq, beta."""
    fact = (g.astype(F32) / np.sqrt(v.astype(F32) + EPS).astype(F32)).astype(F32)
    ws = (w.astype(F32) * fact[:, None, None, None]).astype(F32)
    delta = np.maximum((np.abs(ws).max(axis=(1, 2, 3), keepdims=True) / QMAX).astype(F32), F32(1e-8))
    wq = (np.clip(np.round((ws / delta).astype(F32)), -127, 127) * delta).astype(F32)
    beta = (b.astype(F32) - m.astype(F32) * fact).astype(F32)
    return wq, beta


def _dup2(a):
    """[64,...] -> [128,...] partition-dup along axis 0."""
    return np.concatenate([a, a], axis=0)


def _build_nc():
    nc = bacc.Bacc("TRN2", target_bir_lowering=False, debug=False, num_devices=NCORES)

    xin = nc.dram_tensor("xin", [2, CIN, PX], DT.float32, kind="ExternalInput")
    w1t = nc.dram_tensor("w1t", [2, 128, 128], DT.float32, kind="ExternalInput")   # [kchunk, cin, cout-dup]
    w2d = nc.dram_tensor("w2d", [128, 9, 128], DT.float32, kind="ExternalInput")   # [cin-dup, tap, cout-dup]
    w3d = nc.dram_tensor("w3d", [128, 2, 128], DT.float32, kind="ExternalInput")   # [cin-dup, couthalf, cout]
    b1d = nc.dram_tensor("b1d", [128], DT.float32, kind="ExternalInput")           # beta1 dup
    b2d = nc.dram_tensor("b2d", [128], DT.float32, kind="ExternalInput")
    b3d = nc.dram_tensor("b3d", [256], DT.float32, kind="ExternalInput")           # beta3 (2 chunks of 128)
    outd = nc.dram_tensor("outp", [2, CIN, PX], DT.float32, kind="ExternalOutput")

    with tile.TileContext(nc) as tc:
        _emit(tc, xin, w1t, w2d, w3d, b1d, b2d, b3d, outd)

    nc.compile()
    return nc


def _emit(tc, xin, w1t, w2d, w3d, b1d, b2d, b3d, outd):
    nc = tc.nc
    rg = [list(range(NCORES))]

    sb = tc.alloc_tile_pool(name="sb", bufs=1)
    big = tc.alloc_tile_pool(name="big", bufs=7)       # 13.5KB f32 staging slots
    bfp = tc.alloc_tile_pool(name="bfp", bufs=7)       # bf16 activation slots
    vec = tc.alloc_tile_pool(name="vec", bufs=1)       # small per-channel vectors
    ps1 = tc.alloc_tile_pool(name="ps1", bufs=2, space="PSUM")
    ps2 = tc.alloc_tile_pool(name="ps2", bufs=2, space="PSUM")
    ps3 = tc.alloc_tile_pool(name="ps3", bufs=2, space="PSUM")
    dram = tc.alloc_tile_pool(name="dram", bufs=1, space="DRAM")

    # ---------------- persistent SBUF loads
    # weights + per-channel consts FIRST (small; the first matmul gates on w1sb)
    w1sb = sb.tile([128, 2, 128], DT.float32, name="w1sb", tag="w1sb")
    nc.sync.dma_start(out=w1sb, in_=w1t.rearrange("k c j -> c k j"))
    b1s = vec.tile([128, 1], DT.float32, name="b1s", tag="b1s")
    nc.sync.dma_start(out=b1s, in_=b1d.rearrange("(c o) -> c o", o=1))
    b2s = vec.tile([128, 1], DT.float32, name="b2s", tag="b2s")
    nc.sync.dma_start(out=b2s, in_=b2d.rearrange("(c o) -> c o", o=1))
    b3s = vec.tile([128, 2], DT.float32, name="b3s", tag="b3s")
    nc.sync.dma_start(out=b3s, in_=b3d.rearrange("(h c) -> c h", c=128))
    w2f = sb.tile([128, 9, 128], DT.float32, name="w2f", tag="w2f")
    nc.sync.dma_start(out=w2f, in_=w2d[:, :, :])
    w3f = sb.tile([128, 2, 128], DT.float32, name="w3f", tag="w3f")
    nc.sync.dma_start(out=w3f, in_=w3d[:, :, :])
    # x in half-image chunks: efficient descriptors, early first-band availability
    xsb = [sb.tile([128, 2, PX], DT.float32, name=f"xsb{k}", tag=f"xsb{k}")
           for k in range(2)]
    HALF = PX // 2
    for i in range(2):
        for h in range(2):
            for k in range(2):
                nc.sync.dma_start(
                    out=xsb[k][:, i, HALF * h:HALF * (h + 1)],
                    in_=xin[i, 128 * k:128 * (k + 1), HALF * h:HALF * (h + 1)])
    magic_t = vec.tile([128, 1], DT.float32, name="magic_t", tag="magic_t")
    nc.vector.memset(magic_t, MAGIC)

    # ---------------- warmup collective: absorbs first-call ncfw cost + core skew
    ccw_i = dram.tile([64], DT.float32, name="ccwi", tag="ccwi")
    ccw_o = dram.tile([64 * NCORES], DT.float32, name="ccwo", tag="ccwo", addr_space="Shared")
    nc.sync.dma_start(out=ccw_i[:], in_=b1d[0:64])
    nc.gpsimd.collective_compute(
        "AllGather", AOP.bypass, replica_groups=rg,
        ins=[ccw_i[:]], outs=[ccw_o[:]],
    )

    # ---------------- collective bounce buffers
    cc_in = [dram.tile([64], DT.float32, name="cc1i", tag="cc1i"),
             dram.tile([64], DT.float32, name="cc2i", tag="cc2i"),
             dram.tile([256], DT.float32, name="cc3i", tag="cc3i"),
             dram.tile([256], DT.float32, name="cc4i", tag="cc4i")]
    cc_out = [dram.tile([64 * NCORES], DT.float32, name="cc1o", tag="cc1o", addr_space="Shared"),
              dram.tile([64 * NCORES], DT.float32, name="cc2o", tag="cc2o", addr_space="Shared"),
              dram.tile([256 * NCORES], DT.float32, name="cc3o", tag="cc3o", addr_space="Shared"),
              dram.tile([256 * NCORES], DT.float32, name="cc4o", tag="cc4o", addr_space="Shared")]

    def allgather_max(idx, mloc, nch, ncol):
        """mloc [128, ncol] local per-channel absmax -> returns (d, s) tiles [128, ncol].

        nch: real channel count (64 -> dup halves, 256 -> 2 chunks)."""
        if nch == 64:
            nc.sync.dma_start(out=cc_in[idx][:], in_=mloc[0:64, 0:1].rearrange("c o -> (c o)"))
        else:
            nc.sync.dma_start(out=cc_in[idx].rearrange("(h c) -> c h", c=128), in_=mloc[:, :])
        nc.gpsimd.collective_compute(
            "AllGather", AOP.bypass, replica_groups=rg,
            ins=[cc_in[idx][:]], outs=[cc_out[idx][:]],
        )
        gm = vec.tile([128, ncol, NCORES], DT.float32, name=f"gm{idx}", tag=f"gm{idx}")
        if nch == 64:
            src = cc_out[idx].rearrange("(r o c) -> c o r", c=64, o=1)
            nc.sync.dma_start(out=gm[0:64], in_=src)
            nc.sync.dma_start(out=gm[64:128], in_=src)
        else:
            for h in range(2):
                nc.sync.dma_start(
                    out=gm[:, h, :],
                    in_=cc_out[idx].rearrange("(r h c) -> c h r", c=128, h=2)[:, h, :])
        m = vec.tile([128, ncol], DT.float32, name=f"m{idx}", tag=f"m{idx}")
        nc.vector.reduce_max(out=m, in_=gm, axis=mybir.AxisListType.X)
        d = vec.tile([128, ncol], DT.float32, name=f"d{idx}", tag=f"d{idx}")
        nc.vector.tensor_scalar(out=d, in0=m, scalar1=float(np.float32(1.0) / np.float32(127.0)),
                                scalar2=1e-8, op0=AOP.mult, op1=AOP.max)
        s = vec.tile([128, ncol], DT.float32, name=f"s{idx}", tag=f"s{idx}")
        nc.vector.reciprocal(out=s, in_=d)
        return d, s

    # ================= stage 1: 1x1 conv 256->64(dup), fp32
    t1 = []
    am1 = vec.tile([128, 2], DT.float32, name="am1", tag="am1")
    for i in range(2):
        t1i = big.tile([128, HP, WP], DT.float32, name=f"t1_{i}", tag="bigf32")
        # zero the 1-px border (conv taps read it; ACT/GPSIMD passes keep it 0)
        nc.vector.memset(t1i[:, 0, :], 0.0)
        nc.vector.memset(t1i[:, HP - 1, :], 0.0)
        nc.vector.memset(t1i[:, 1:HP - 1, 0:1], 0.0)
        nc.vector.memset(t1i[:, 1:HP - 1, WP - 1:WP], 0.0)
        for b in range(NB):
            ps = ps1.tile([128, 512], DT.float32, name="ps1t", tag="ps1t")
            for k in range(2):
                nc.tensor.matmul(ps[:, 0:BAND], w1sb[:, k, :],
                                 xsb[k][:, i, BAND * b:BAND * (b + 1)],
                                 start=(k == 0), stop=(k == 1))
            nc.vector.tensor_scalar(
                out=t1i[:, 1 + 8 * b:9 + 8 * b, 1:57],
                in0=ps[:, 0:BAND].rearrange("c (r w) -> c r w", r=8),
                scalar1=b1s, scalar2=None, op0=AOP.add)
        nc.vector.tensor_reduce(out=am1[:, i:i + 1], in_=t1i,
                                axis=mybir.AxisListType.XY, op=AOP.max,
                                apply_absolute_value=True)
        t1.append(t1i)
    m1loc = vec.tile([128, 1], DT.float32, name="m1loc", tag="m1loc")
    nc.vector.reduce_max(out=m1loc, in_=am1, axis=mybir.AxisListType.X)
    d1, s1 = allgather_max(0, m1loc, 64, 1)

    # fold + split stage-2 weights: W2' = w2f * d1[cin]; hi=bf16(W2'), lo=bf16(W2'-hi)
    w2hi = sb.tile([128, 9, 128], DT.bfloat16, name="w2hi", tag="w2hi")
    nc.vector.tensor_scalar(out=w2hi, in0=w2f, scalar1=d1, scalar2=None, op0=AOP.mult)
    p2 = sb.tile([128, 9, 128], DT.bfloat16, name="p2", tag="p2")
    nc.vector.tensor_copy(out=p2[0:64], in_=w2hi[0:64])
    nc.vector.scalar_tensor_tensor(out=p2[64:128], in0=w2f[64:128], scalar=d1[64:128],
                                   in1=w2hi[64:128], op0=AOP.mult, op1=AOP.subtract)

    # a1 = relu(round(t1*s1)) as integer-valued bf16, padded+dup layout
    a1 = []
    for i in range(2):
        nc.scalar.activation(out=t1[i][:], in_=t1[i][:], func=AF.Identity, bias=magic_t, scale=s1)
        a1i = bfp.tile([128, HP, WP], DT.bfloat16, name=f"a1_{i}", tag="bfact")
        nc.vector.tensor_scalar(out=a1i, in0=t1[i][:], scalar1=MAGIC, scalar2=0.0,
                                op0=AOP.subtract, op1=AOP.max)
        a1.append(a1i)

    # ================= stage 2: 3x3 conv 64->64(dup), bf16 hi/lo packed K=128
    t2 = []
    am2 = vec.tile([128, 2], DT.float32, name="am2", tag="am2")
    for i in range(2):
        t2i = big.tile([128, PX], DT.float32, name=f"t2_{i}", tag="bigf32")
        for bt in range(4):  # psum tiles of 2 bands (last has 1)
            bands = [2 * bt, 2 * bt + 1] if bt < 3 else [6]
            ps = ps2.tile([128, 2, 512], DT.float32, name="ps2t", tag="ps2t")
            for tap in range(9):
                dy, dx = tap // 3, tap % 3
                for j, b in enumerate(bands):
                    nc.tensor.matmul(ps[:, j, 0:BAND], p2[:, tap, :],
                                     a1[i][:, 8 * b + dy:8 * b + dy + 8, dx:dx + 56],
                                     start=(tap == 0), stop=(tap == 8))
            for j, b in enumerate(bands):
                nc.vector.tensor_scalar(
                    out=t2i[:, BAND * b:BAND * (b + 1)],
                    in0=ps[:, j, 0:BAND],
                    scalar1=b2s, scalar2=None, op0=AOP.add)
        nc.vector.tensor_reduce(out=am2[:, i:i + 1], in_=t2i,
                                axis=mybir.AxisListType.X, op=AOP.max,
                                apply_absolute_value=True)
        t2.append(t2i)
    m2loc = vec.tile([128, 1], DT.float32, name="m2loc", tag="m2loc")
    nc.vector.reduce_max(out=m2loc, in_=am2, axis=mybir.AxisListType.X)
    d2, s2 = allgather_max(1, m2loc, 64, 1)

    # fold + split stage-3 weights
    w3hi = sb.tile([128, 2, 128], DT.bfloat16, name="w3hi", tag="w3hi")
    nc.vector.tensor_scalar(out=w3hi, in0=w3f, scalar1=d2, scalar2=None, op0=AOP.mult)
    p3 = sb.tile([128, 2, 128], DT.bfloat16, name="p3", tag="p3")
    nc.vector.tensor_copy(out=p3[0:64], in_=w3hi[0:64])
    nc.vector.scalar_tensor_tensor(out=p3[64:128], in0=w3f[64:128], scalar=d2[64:128],
                                   in1=w3hi[64:128], op0=AOP.mult, op1=AOP.subtract)

    a2 = []
    for i in range(2):
        nc.scalar.activation(out=t2[i][:], in_=t2[i][:], func=AF.Identity, bias=magic_t, scale=s2)
        a2i = bfp.tile([128, PX], DT.bfloat16, name=f"a2_{i}", tag="bfact")
        nc.vector.tensor_scalar(out=a2i, in0=t2[i][:], scalar1=MAGIC, scalar2=0.0,
                                op0=AOP.subtract, op1=AOP.max)
        a2.append(a2i)

    # ================= stage 3: 1x1 conv 64->256 (2 chunks of 128), bf16 packed
    t3 = [[None, None], [None, None]]
    am3 = vec.tile([128, 2, 2], DT.float32, name="am3", tag="am3")  # [c, i]
    for i in range(2):
        for c in range(2):
            t3ic = big.tile([128, PX], DT.float32, name=f"t3_{i}{c}", tag="bigf32")
            for b in range(NB):
                ps = ps3.tile([128, 512], DT.float32, name="ps3t", tag="ps3t")
                nc.tensor.matmul(ps[:, 0:BAND], p3[:, c, :],
                                 a2[i][:, BAND * b:BAND * (b + 1)],
                                 start=True, stop=True)
                nc.scalar.activation(out=t3ic[:, BAND * b:BAND * (b + 1)], in_=ps[:, 0:BAND],
                                     func=AF.Identity, bias=b3s[:, c:c + 1], scale=1.0)
            nc.vector.tensor_reduce(out=am3[:, c, i:i + 1], in_=t3ic, axis=mybir.AxisListType.X,
                                    op=AOP.max, apply_absolute_value=True)
            t3[i][c] = t3ic
    m3loc = vec.tile([128, 2], DT.float32, name="m3loc", tag="m3loc")
    nc.vector.reduce_max(out=m3loc, in_=am3, axis=mybir.AxisListType.X)
    d3, s3 = allgather_max(2, m3loc, 256, 2)

    # ================= residual + final quant
    z = [[None, None], [None, None]]
    am4 = vec.tile([128, 2, 2], DT.float32, name="am4", tag="am4")
    for i in range(2):
        for c in range(2):
            t3ic = t3[i][c]
            nc.scalar.activation(out=t3ic[:], in_=t3ic[:], func=AF.Identity,
                                 bias=magic_t, scale=s3[:, c:c + 1])
            q3d = big.tile([128, PX], DT.float32, name=f"q3d_{i}{c}", tag="bigf32")
            nc.vector.tensor_scalar(out=q3d, in0=t3ic, scalar1=MAGIC,
                                    scalar2=d3[:, c:c + 1],
                                    op0=AOP.subtract, op1=AOP.mult)
            zic = q3d
            nc.vector.tensor_add(out=zic[:], in0=q3d[:], in1=xsb[c][:, i, :])
            nc.vector.tensor_reduce(out=am4[:, c, i:i + 1], in_=zic,
                                    axis=mybir.AxisListType.X, op=AOP.max,
                                    apply_absolute_value=True)
            z[i][c] = zic
    m4loc = vec.tile([128, 2], DT.float32, name="m4loc", tag="m4loc")
    nc.vector.reduce_max(out=m4loc, in_=am4, axis=mybir.AxisListType.X)
    d4, s4 = allgather_max(3, m4loc, 256, 2)

    for i in range(2):
        for c in range(2):
            zic = z[i][c]
            nc.scalar.activation(out=zic[:], in_=zic[:], func=AF.Relu,
                                 bias=0.0, scale=s4[:, c:c + 1])
            nc.vector.tensor_scalar(out=zic[:], in0=zic[:], scalar1=MAGIC, scalar2=MAGIC,
                                    op0=AOP.add, op1=AOP.subtract)
            nc.vector.tensor_scalar(out=zic[:], in0=zic[:], scalar1=d4[:, c:c + 1],
                                    scalar2=None, op0=AOP.mult)
            nc.sync.dma_start(out=outd[i, 128 * c:128 * (c + 1), :], in_=zic[:])

    for p in (dram, ps3, ps2, ps1, vec, bfp, big, sb):
        p.release()


_NC_CACHE = {}


def _get_nc():
    if "nc" not in _NC_CACHE:
        _NC_CACHE["nc"] = _build_nc()
    return _NC_CACHE["nc"]


def kernel(x, w1, g1, b1, m1, v1, w2, g2, b2, m2, v2, w3, g3, b3, m3, v3,
           _want_profile=False):
    x = np.ascontiguousarray(x, dtype=F32)

    wq1, beta1 = _host_fold(w1, g1, b1, m1, v1)
    wq2, beta2 = _host_fold(w2, g2, b2, m2, v2)
    wq3, beta3 = _host_fold(w3, g3, b3, m3, v3)

    # stage1 lhsT [kchunk, cin(128), cout-dup(128)]
    w1m = wq1[:, :, 0, 0]                       # [64, 256]
    w1t = np.stack([w1m[:, 0:128].T, w1m[:, 128:256].T], axis=0)       # [2,128,64]
    w1t = np.concatenate([w1t, w1t], axis=2).astype(F32)               # [2,128,128] cout-dup

    # stage2 [cin-dup(128), tap(9), cout-dup(128)]
    w2r = wq2[:, :, :, :].reshape(64, 64, 9).transpose(1, 2, 0)        # [cin, tap, cout]
    w2dn = np.concatenate([w2r, w2r], axis=0)                          # cin-dup
    w2dn = np.concatenate([w2dn, w2dn], axis=2).astype(F32)            # cout-dup

    # stage3 [cin-dup(128), couthalf(2), cout(128)]
    w3r = wq3[:, :, 0, 0].T                                            # [64, 256]
    w3dn = np.stack([w3r[:, 0:128], w3r[:, 128:256]], axis=1)          # [64, 2, 128]
    w3dn = np.concatenate([w3dn, w3dn], axis=0).astype(F32)            # cin-dup

    b1dn = _dup2(beta1).astype(F32)
    b2dn = _dup2(beta2).astype(F32)
    b3dn = beta3.astype(F32)                                           # [256]

    nc = _get_nc()
    in_maps = []
    for c in range(NCORES):
        in_maps.append({
            "xin": np.ascontiguousarray(x[2 * c:2 * c + 2].reshape(2, CIN, PX)),
            "w1t": w1t, "w2d": w2dn, "w3d": w3dn,
            "b1d": b1dn, "b2d": b2dn, "b3d": b3dn,
        })
    res = run_bass_kernel_spmd(nc, in_maps, list(range(NCORES)), trace=_want_profile)
    out = np.empty((N, CIN, PX), dtype=F32)
    for c in range(NCORES):
        out[2 * c:2 * c + 2] = res.results[c]["outp"]
    out = out.reshape(N, CIN, H, W)
    if _want_profile:
        return out, res
    return out
